# revision 4
# baseline (speedup 1.0000x reference)
"""DenseShift forward kernel for Trainium2 (8 NeuronCores, data-parallel).

Computes y = x @ W + bias where W = 2^shift * (-1)^sign, for
x: [524288, 256] f32, shift/sign: [256, 256], bias: [1, 256].

Sharding: x is split along batch across 8 cores (65536 rows each);
shift/sign/bias are replicated. No collectives (forward only).

Per-core dataflow (memory-bound problem; the point is streaming x/y at
HBM rate while the PE keeps up):
  - W is reconstructed exactly on-device with integer bit ops:
    bits = ((shift + 127) << 23) | (sign << 31), bitcast to f32.
  - x is DMA'd in 2 MiB groups (16 b-tiles of [128, 256]).
  - Each [128, 128] chunk of x is transposed on the PE (is_transpose
    passthrough, exact) into PSUM, then moved to SBUF by the DVE.
  - Matmul precision tiers:
      "tf32":   xT rounded to float32r (TF32), 2 matmuls per b-tile.
      "tf32x2": hi/lo TF32 split of xT (Kahan residual), 4 matmuls —
                ~2^-22 relative accuracy at half the cost of fp32.
      "fp32":   plain fp32 matmuls (4 cycles/row on the PE).
    W entries are powers of two, exact in every tier.
  - bias add is fused into the mandatory PSUM->SBUF DVE copy of y.
  - y written back in 2 MiB groups.
"""

import numpy as np

import concourse.mybir as mybir
import concourse.tile as tile
from concourse import bacc
from concourse.bass_utils import run_bass_kernel_spmd
from concourse.masks import make_identity

N_CORES = 8
BATCH, IN_DIM, OUT_DIM = 524288, 256, 256
B_CORE = BATCH // N_CORES  # 65536 rows per core
PRECISION = "tf32x2"

F32 = mybir.dt.float32
F32R = mybir.dt.float32r
I32 = mybir.dt.int32


def build_bass(
    b_core: int = B_CORE, group_tiles: int = 16, precision: str = PRECISION
) -> "bacc.Bacc":
    """Build the per-core SPMD Bass program."""
    P = 128
    G = group_tiles
    assert b_core % (P * G) == 0
    n_groups = b_core // (P * G)
    mm_dt = F32 if precision == "fp32" else F32R

    nc = bacc.Bacc(
        "TRN2", target_bir_lowering=False, debug=False, num_devices=N_CORES
    )
    x = nc.dram_tensor("x", [b_core, IN_DIM], F32, kind="ExternalInput").ap()
    shift = nc.dram_tensor("shift", [IN_DIM, OUT_DIM], F32, kind="ExternalInput").ap()
    sign = nc.dram_tensor("sign", [IN_DIM, OUT_DIM], F32, kind="ExternalInput").ap()
    bias = nc.dram_tensor("bias", [1, OUT_DIM], F32, kind="ExternalInput").ap()
    y = nc.dram_tensor("y", [b_core, OUT_DIM], F32, kind="ExternalOutput").ap()

    # [g, p, t, m] views: group g covers rows [g*G*128, (g+1)*G*128)
    x_v = x.rearrange("(g t p) m -> g p t m", t=G, p=P)
    y_v = y.rearrange("(g t p) m -> g p t m", t=G, p=P)

    with tile.TileContext(nc) as tc:
        with (
            tc.tile_pool(name="const", bufs=1) as const_pool,
            tc.tile_pool(name="xin", bufs=3) as in_pool,
            tc.tile_pool(name="yout", bufs=3) as out_pool,
            tc.tile_pool(name="xt", bufs=4) as xt_pool,
            tc.tile_pool(name="pst", bufs=3, space="PSUM") as psum_t_pool,
            tc.tile_pool(name="psy", bufs=3, space="PSUM") as psum_y_pool,
        ):
            # ---- constants ----
            ident = const_pool.tile([P, P], F32)
            make_identity(nc, ident[:])

            # W = 2^shift * (-1)^sign, exactly, via exponent-field bits.
            # Layout: w[:, c*256:(c+1)*256] = W[c*128:(c+1)*128, :]
            sh = const_pool.tile([P, 2 * OUT_DIM], F32)
            sg = const_pool.tile([P, 2 * OUT_DIM], F32)
            for c in range(2):
                cs = slice(c * OUT_DIM, (c + 1) * OUT_DIM)
                rs = slice(c * P, (c + 1) * P)
                nc.sync.dma_start(sh[:, cs], shift[rs, :])
                nc.sync.dma_start(sg[:, cs], sign[rs, :])
            sh_i = const_pool.tile([P, 2 * OUT_DIM], I32)
            sg_i = const_pool.tile([P, 2 * OUT_DIM], I32)
            w_i = const_pool.tile([P, 2 * OUT_DIM], I32)
            # biased exponent (shift + 127), still f32 -> int32 (exact ints)
            nc.vector.tensor_scalar_add(sh[:], sh[:], 127.0)
            nc.vector.tensor_copy(sh_i[:], sh[:])
            nc.vector.tensor_copy(sg_i[:], sg[:])
            nc.vector.tensor_scalar(
                sh_i[:], sh_i[:], 23, None, op0=mybir.AluOpType.logical_shift_left
            )
            nc.vector.tensor_scalar(
                sg_i[:], sg_i[:], 31, None, op0=mybir.AluOpType.logical_shift_left
            )
            nc.vector.tensor_tensor(
                w_i[:], sh_i[:], sg_i[:], op=mybir.AluOpType.bitwise_or
            )
            # materialize W at the matmul dtype (values are powers of two,
            # exact under TF32 rounding)
            w_mm = const_pool.tile([P, 2 * OUT_DIM], mm_dt)
            nc.vector.tensor_copy(w_mm[:], w_i[:].bitcast(F32))

            # bias broadcast to all 128 partitions via a K=1 matmul of
            # ones[1,128].T @ bias[1,256]
            ones = const_pool.tile([1, P], F32)
            nc.gpsimd.memset(ones[:], 1.0)
            bias_row = const_pool.tile([1, OUT_DIM], F32)
            nc.sync.dma_start(bias_row[:], bias[:])
            bias_bc = const_pool.tile([P, OUT_DIM], F32)
            psum_b = psum_t_pool.tile([P, OUT_DIM], F32, tag="ps_t")
            nc.tensor.matmul(psum_b[:], ones[:], bias_row[:], start=True, stop=True)
            nc.vector.tensor_copy(bias_bc[:], psum_b[:])

            # ---- main loop ----
            for g in range(n_groups):
                x_in = in_pool.tile([P, G, IN_DIM], F32)
                nc.sync.dma_start(x_in[:], x_v[g])
                y_out = out_pool.tile([P, G, OUT_DIM], F32)
                for t in range(G):
                    # transpose both 128-chunks of x tile into one PSUM bank
                    ps_t = psum_t_pool.tile([P, IN_DIM], F32, tag="ps_t")
                    for c in range(2):
                        nc.tensor.transpose(
                            ps_t[:, c * P : (c + 1) * P],
                            x_in[:, t, c * P : (c + 1) * P],
                            ident[:],
                        )
                    xT = xt_pool.tile([P, IN_DIM], mm_dt, tag="xt_hi")
                    nc.vector.tensor_copy(xT[:], ps_t[:])
                    ps_y = psum_y_pool.tile([P, OUT_DIM], F32)
                    if precision == "tf32x2":
                        xT_lo = xt_pool.tile([P, IN_DIM], F32R, tag="xt_lo")
                        nc.vector.tensor_tensor(
                            xT_lo[:], ps_t[:], xT[:], op=mybir.AluOpType.subtract
                        )
                        parts = [(xT, 0), (xT, 1), (xT_lo, 0), (xT_lo, 1)]
                    else:
                        parts = [(xT, 0), (xT, 1)]
                    for i, (src, c) in enumerate(parts):
                        nc.tensor.matmul(
                            ps_y[:],
                            src[:, c * P : (c + 1) * P],
                            w_mm[:, c * OUT_DIM : (c + 1) * OUT_DIM],
                            start=(i == 0),
                            stop=(i == len(parts) - 1),
                        )
                    # fused bias-add + PSUM->SBUF move
                    nc.vector.tensor_add(y_out[:, t, :], ps_y[:], bias_bc[:])
                nc.sync.dma_start(y_v[g], y_out[:])
    nc.compile()
    return nc


_NC_CACHE: dict = {}


def _get_nc():
    if "nc" not in _NC_CACHE:
        _NC_CACHE["nc"] = build_bass()
    return _NC_CACHE["nc"]


def kernel(x, shift, sign, bias):
    x = np.ascontiguousarray(x, dtype=np.float32)
    shift = np.ascontiguousarray(shift, dtype=np.float32)
    sign = np.ascontiguousarray(sign, dtype=np.float32)
    bias = np.ascontiguousarray(bias, dtype=np.float32)
    assert x.shape == (BATCH, IN_DIM)

    nc = _get_nc()
    shards = np.split(x, N_CORES, axis=0)
    in_maps = [
        {"x": shards[c], "shift": shift, "sign": sign, "bias": bias}
        for c in range(N_CORES)
    ]
    res = run_bass_kernel_spmd(nc, in_maps, core_ids=list(range(N_CORES)))
    return np.concatenate([r["y"] for r in res.results], axis=0)
